# revision 14
# baseline (speedup 1.0000x reference)
"""Trainium2 Bass kernel for a causal transformer decoder block.

Sharding: sequence-parallel, no collectives. 8 cores = 2 batches x 4
query-chunk groups. Core c (j = c % 4, b = c // 4) handles batch b and two
256-row query chunks: rows [256j, 256j+256) and [256(7-j), 256(8-j)).
Chunk A runs attention against keys [0, 1024), chunk B against [0, 2048)
(padded to a uniform extent so all 8 cores execute one SPMD program; the
causal mask is applied via host-precomputed additive bias tiles, so the
padding is data, not control flow). Every core computes full-sequence K/V
for its batch; weights are replicated. Matmuls run in float32r (full-rate
fp32 PE streaming mode) with fp32 PSUM accumulation.
"""

import os
import sys

for _p in ("/opt/trn_rl_repo", "/root/.axon_site/_ro/trn_rl_repo"):
    if os.path.isdir(_p) and _p not in sys.path:
        sys.path.insert(0, _p)

import numpy as np

import concourse.bacc as bacc
import concourse.bass as bass
import concourse.mybir as mybir
import concourse.tile as tile
from concourse.bass_utils import run_bass_kernel_spmd

P = 128
B, S, D = 2, 2048, 768
H, DK = 12, 64
DFF = 3072
NCORES = 8
CH = 256  # query chunk rows per chunk (2 chunks per core)
QR = 2 * CH  # query rows per core
EXT = (1024, 2048)  # padded key extents for chunk A / chunk B
KB = D // P  # 6 contraction blocks over D
MB = DFF // P  # 24 blocks over DFF
NEG = -8.0e9  # raw-score mask offset; * 0.125 -> -1e9 before exp

F32 = mybir.dt.float32
F32R = mybir.dt.float32r
AF = mybir.ActivationFunctionType

_PROGRAM = None


def _bcast_row(nc, dram_ap, n):
    """AP view of a [n] DRAM vector broadcast to [P, n]."""
    return bass.AP(tensor=dram_ap.tensor, offset=dram_ap.offset, ap=[[0, P], [1, n]])


def _nsplit(n):
    """Split a free dim into <=512 chunks (each >=256 for f32r full rate)."""
    out, s = [], 0
    while n - s > 512:
        out.append((s, 512))
        s += 512
    out.append((s, n - s))
    return out


def build_program_for_sim():
    return build_program(finalize=False)


def build_program(finalize=True):
    nc = bacc.Bacc(None)

    xb = nc.declare_dram_parameter("xb", [S, D], F32R, isOutput=False)
    xq = nc.declare_dram_parameter("xq", [QR, D], F32R, isOutput=False)
    xr = nc.declare_dram_parameter("xr", [QR, D], F32, isOutput=False)
    wq = nc.declare_dram_parameter("wq", [D, D], F32R, isOutput=False)
    wk = nc.declare_dram_parameter("wk", [D, D], F32R, isOutput=False)
    wv = nc.declare_dram_parameter("wv", [D, D], F32R, isOutput=False)
    wo = nc.declare_dram_parameter("wo", [D, D], F32R, isOutput=False)
    w1 = nc.declare_dram_parameter("w1", [D, DFF], F32R, isOutput=False)
    w2 = nc.declare_dram_parameter("w2", [DFF, D], F32R, isOutput=False)
    bq = nc.declare_dram_parameter("bq", [D], F32, isOutput=False)
    bk = nc.declare_dram_parameter("bk", [D], F32, isOutput=False)
    b1 = nc.declare_dram_parameter("b1", [DFF], F32, isOutput=False)
    b2 = nc.declare_dram_parameter("b2", [D], F32, isOutput=False)
    g1 = nc.declare_dram_parameter("g1", [D], F32, isOutput=False)
    bl1 = nc.declare_dram_parameter("bl1", [D], F32, isOutput=False)
    g2 = nc.declare_dram_parameter("g2", [D], F32, isOutput=False)
    bl2 = nc.declare_dram_parameter("bl2", [D], F32, isOutput=False)
    ident_d = nc.declare_dram_parameter("ident", [P, P], F32R, isOutput=False)
    ones_d = nc.declare_dram_parameter("ones64", [DK], F32R, isOutput=False)
    vones_d = nc.declare_dram_parameter("vones", [H], F32R, isOutput=False)
    biasA = nc.declare_dram_parameter("biasA", [EXT[0], CH], F32, isOutput=False)
    biasB = nc.declare_dram_parameter("biasB", [EXT[1], CH], F32, isOutput=False)
    out = nc.declare_dram_parameter("out", [QR, D], F32, isOutput=True)

    with tile.TileContext(nc) as tc:
        _emit(nc, tc, locals())
    if finalize:
        if not nc.is_finalized():
            nc.finalize()
    else:
        nc.compile()
    return nc


def _layernorm(nc, t, gb, bb, eps_t, stats_pool):
    """In-place LayerNorm of SBUF tile t [P, D] over the free dim."""
    sub = 256  # gcd(512, 768)
    nsub = D // sub
    stats = stats_pool.tile([P, nsub, 6], F32, tag="ln_stats")
    tv = t.rearrange("p (n s) -> p n s", s=sub)
    for i in range(nsub):
        nc.vector.bn_stats(out=stats[:, i, :], in_=tv[:, i, :])
    mv = stats_pool.tile([P, 2], F32, tag="ln_mv")
    nc.vector.bn_aggr(out=mv, in_=stats)
    # mv[:,1] = 1/sqrt(var + eps)
    nc.scalar.activation(
        out=mv[:, 1:2], in_=mv[:, 1:2], func=AF.Sqrt, bias=eps_t, scale=1.0
    )
    nc.vector.reciprocal(out=mv[:, 1:2], in_=mv[:, 1:2])
    nc.vector.tensor_scalar(
        out=t,
        in0=t,
        scalar1=mv[:, 0:1],
        scalar2=mv[:, 1:2],
        op0=mybir.AluOpType.subtract,
        op1=mybir.AluOpType.mult,
    )
    nc.vector.tensor_mul(out=t, in0=t, in1=gb)
    nc.vector.tensor_add(out=t, in0=t, in1=bb)


def _emit(nc, tc, io):
    xb, xq, xr = io["xb"], io["xq"], io["xr"]
    ident_d, ones_d, vones_d = io["ident_d"], io["ones_d"], io["vones_d"]
    wq, wk, wv, wo, w1, w2 = (io[k] for k in ("wq", "wk", "wv", "wo", "w1", "w2"))
    bq, bk, b1, b2 = io["bq"], io["bk"], io["b1"], io["b2"]
    g1, bl1, g2, bl2 = io["g1"], io["bl1"], io["g2"], io["bl2"]
    biasA, biasB, out = io["biasA"], io["biasB"], io["out"]

    from contextlib import ExitStack

    es = ExitStack()
    const = es.enter_context(tc.tile_pool(name="const", bufs=1))
    ident = const.tile([P, P], F32R)
    nc.sync.dma_start(out=ident, in_=ident_d[:, :])
    eps_t = const.tile([P, 1], F32)
    nc.vector.memset(eps_t, 1e-5)
    ones_t = const.tile([P, DK], F32R)
    nc.sync.dma_start(out=ones_t, in_=_bcast_row(nc, ones_d[:], DK))
    vones_t = const.tile([P, H], F32R)
    nc.sync.dma_start(out=vones_t, in_=_bcast_row(nc, vones_d[:], H))
    bq_t = const.tile([P, KB], F32)
    nc.gpsimd.dma_start(out=bq_t, in_=bq[:].rearrange("(k p) -> p k", p=P))
    bk_t = const.tile([P, KB], F32)
    nc.gpsimd.dma_start(out=bk_t, in_=bk[:].rearrange("(k p) -> p k", p=P))
    b1_t = const.tile([P, MB], F32)
    nc.gpsimd.dma_start(out=b1_t, in_=b1[:].rearrange("(m p) -> p m", p=P))
    g1_t = const.tile([P, D], F32)
    nc.sync.dma_start(out=g1_t, in_=_bcast_row(nc, g1[:], D))
    bl1_t = const.tile([P, D], F32)
    nc.sync.dma_start(out=bl1_t, in_=_bcast_row(nc, bl1[:], D))
    g2_t = const.tile([P, D], F32)
    nc.sync.dma_start(out=g2_t, in_=_bcast_row(nc, g2[:], D))
    bl2_t = const.tile([P, D], F32)
    nc.sync.dma_start(out=bl2_t, in_=_bcast_row(nc, bl2[:], D))
    b2_t = const.tile([P, D], F32)
    nc.sync.dma_start(out=b2_t, in_=_bcast_row(nc, b2[:], D))
    xr_t = const.tile([P, QR // P, D], F32)
    nc.sync.dma_start(out=xr_t, in_=xr[:, :].rearrange("(qb p) d -> p qb d", p=P))
    x1_t = const.tile([P, QR // P, D], F32R)  # post-LN1 activations

    with (
        tc.tile_pool(name="persist", bufs=1) as persist,
    ):
        kT = persist.tile([P, KB, S], F32R)  # K^T: [d within pb, pb, key]
        qT = persist.tile([P, KB, QR], F32R)  # Q^T
        # V with a ones column per head: [k within block, key block, h*65+e]
        vA = persist.tile([P, S // P, H * (DK + 1)], F32R)
        vA4 = vA.rearrange("p s (h e) -> p s h e", e=DK + 1)
        for s in range(S // P):
            nc.vector.tensor_copy(out=vA4[:, s, :, DK], in_=vones_t)

        # ---------------- Phase 1: QKV projections ----------------
        with (
            tc.tile_pool(name="qkv_x", bufs=3) as xpool,
            tc.tile_pool(name="qkv_ps", bufs=2, space="PSUM") as pps,
            tc.tile_pool(name="qkv_tps", bufs=2, space="PSUM") as tps,
        ):

            def transpose_chunk(src_dram, row0, xt_tile, col0, ncols):
                for sb in range(ncols // P):
                    xtile = xpool.tile([P, D], F32R, tag="xload", name="xload")
                    nc.sync.dma_start(
                        out=xtile,
                        in_=src_dram[row0 + sb * P : row0 + (sb + 1) * P, :],
                    )
                    for k in range(KB):
                        tp = tps.tile([P, P], F32R, tag="tps", name="tps")
                        nc.tensor.transpose(tp, xtile[:, k * P : (k + 1) * P], ident)
                        nc.vector.tensor_copy(
                            out=xt_tile[:, k, col0 + sb * P : col0 + (sb + 1) * P],
                            in_=tp,
                        )

            # Q^T from xq
            with (
                tc.tile_pool(name="q_w", bufs=1) as wqpool,
                tc.tile_pool(name="q_xt", bufs=1) as xqtpool,
            ):
                wq_t = [
                    wqpool.tile([P, D], F32R, tag=f"wq{k}", name=f"wq{k}")
                    for k in range(KB)
                ]
                for k in range(KB):
                    nc.sync.dma_start(out=wq_t[k], in_=wq[k * P : (k + 1) * P, :])
                xqT = xqtpool.tile([P, KB, QR], F32R, tag="xqT", name="xqT")
                transpose_chunk(xq, 0, xqT, 0, QR)
                for pb in range(KB):
                    for n0, nn in _nsplit(QR):
                        ps = pps.tile([P, 512], F32, tag="proj", name="proj")
                        for k in range(KB):
                            nc.tensor.matmul(
                                ps[:, :nn],
                                lhsT=wq_t[k][:, pb * P : (pb + 1) * P],
                                rhs=xqT[:, k, n0 : n0 + nn],
                                start=(k == 0),
                                stop=(k == KB - 1),
                            )
                        nc.vector.tensor_scalar_add(
                            out=qT[:, pb, n0 : n0 + nn],
                            in0=ps[:, :nn],
                            scalar1=bq_t[:, pb : pb + 1],
                        )

            # K^T and V per 512-row key chunk
            with (
                tc.tile_pool(name="kv_w", bufs=1) as wkvpool,
                tc.tile_pool(name="kv_xt", bufs=1) as xbtpool,
            ):
                wk_t = [
                    wkvpool.tile([P, D], F32R, tag=f"wk{k}", name=f"wk{k}")
                    for k in range(KB)
                ]
                wv_t = [
                    wkvpool.tile([P, D], F32R, tag=f"wv{k}", name=f"wv{k}")
                    for k in range(KB)
                ]
                for k in range(KB):
                    nc.sync.dma_start(out=wk_t[k], in_=wk[k * P : (k + 1) * P, :])
                    nc.sync.dma_start(out=wv_t[k], in_=wv[k * P : (k + 1) * P, :])
                for sc in range(S // 512):
                    xbT = xbtpool.tile([P, KB, 512], F32R, tag="xbT", name="xbT")
                    transpose_chunk(xb, sc * 512, xbT, 0, 512)
                    for pb in range(KB):
                        ps = pps.tile([P, 512], F32, tag="proj", name="proj")
                        for k in range(KB):
                            nc.tensor.matmul(
                                ps,
                                lhsT=wk_t[k][:, pb * P : (pb + 1) * P],
                                rhs=xbT[:, k, :],
                                start=(k == 0),
                                stop=(k == KB - 1),
                            )
                        nc.vector.tensor_scalar_add(
                            out=kT[:, pb, sc * 512 : (sc + 1) * 512],
                            in0=ps,
                            scalar1=bk_t[:, pb : pb + 1],
                        )
                    for sb in range(4):
                        ps = pps.tile([P, D], F32, tag="vproj", name="vproj")
                        for k in range(KB):
                            for n0, nn in _nsplit(D):
                                nc.tensor.matmul(
                                    ps[:, n0 : n0 + nn],
                                    lhsT=xbT[:, k, sb * P : (sb + 1) * P],
                                    rhs=wv_t[k][:, n0 : n0 + nn],
                                    start=(k == 0),
                                    stop=(k == KB - 1),
                                )
                        nc.vector.tensor_copy(
                            out=vA4[:, sc * 4 + sb, :, 0:DK],
                            in_=ps.rearrange("p (h d) -> p h d", d=DK),
                        )

        # ---------------- Phase 2: attention ----------------
        with (
            tc.tile_pool(name="att_wo", bufs=1) as wopool,
            tc.tile_pool(name="att_bias", bufs=1) as bpool,
            tc.tile_pool(name="att_p", bufs=3) as ppool,
            tc.tile_pool(name="att_ctx", bufs=2) as cpool,
            tc.tile_pool(name="att_dn", bufs=2) as dpool,
            tc.tile_pool(name="att_st", bufs=2, space="PSUM") as stps,
            tc.tile_pool(name="att_cx", bufs=2, space="PSUM") as cxps,
            tc.tile_pool(name="att_bc", bufs=1, space="PSUM") as bcps,
            tc.tile_pool(name="att_o", bufs=1, space="PSUM") as ops,
            tc.tile_pool(name="ln_stats", bufs=3) as spool,
        ):
            wo_t = [
                wopool.tile([P, D], F32R, tag=f"wo{k}", name=f"wo{k}")
                for k in range(KB)
            ]
            for k in range(KB):
                nc.sync.dma_start(out=wo_t[k], in_=wo[k * P : (k + 1) * P, :])

            for ch, (bias_d, ext) in enumerate(zip((biasA, biasB), EXT)):
                nkb = ext // P
                bias_t = bpool.tile(
                    [P, EXT[1] // P, CH], F32, tag="bias", name="bias"
                )[:, :nkb, :]
                nc.sync.dma_start(
                    out=bias_t,
                    in_=bias_d[:, :].rearrange("(n p) q -> p n q", p=P),
                )
                ctx = cpool.tile([P, KB, CH], F32R, tag="ctx", name="ctx")
                for h in range(H):
                    pb, base = h // 2, DK * (h % 2)
                    cx = cxps.tile([DK + 1, CH], F32, tag="cx", name="cx")
                    for kb in range(nkb):
                        st = stps.tile([P, CH], F32, tag="st", name="st")
                        nc.tensor.matmul(
                            st,
                            lhsT=kT[base : base + DK, pb, kb * P : (kb + 1) * P],
                            rhs=qT[base : base + DK, pb, ch * CH : (ch + 1) * CH],
                            start=True,
                            stop=True,
                        )
                        nc.vector.tensor_add(out=st, in0=st, in1=bias_t[:, kb, :])
                        pt = ppool.tile([P, CH], F32R, tag="pt", name="pt")
                        nc.scalar.activation(out=pt, in_=st, func=AF.Exp, scale=0.125)
                        nc.tensor.matmul(
                            cx,
                            lhsT=vA[:, kb, h * (DK + 1) : (h + 1) * (DK + 1)],
                            rhs=pt,
                            start=(kb == 0),
                            stop=(kb == nkb - 1),
                        )
                    # denominator row -> reciprocal broadcast -> normalized ctx^T
                    dn = dpool.tile([P, CH], F32R, tag="dn", name="dn")
                    nc.vector.tensor_copy(
                        out=dn[DK : DK + 1, :], in_=cx[DK : DK + 1, :]
                    )
                    bc = bcps.tile([DK, CH], F32, tag="bc", name="bc")
                    nc.tensor.matmul(
                        bc,
                        lhsT=ones_t[DK : DK + 1, :],
                        rhs=dn[DK : DK + 1, :],
                        start=True,
                        stop=True,
                    )
                    rc = dpool.tile([DK, CH], F32, tag="rc", name="rc")
                    nc.vector.reciprocal(out=rc, in_=bc)
                    nc.vector.tensor_mul(
                        out=ctx[base : base + DK, pb, :], in0=cx[0:DK, :], in1=rc
                    )
                # O-projection + residual + LN1 per 128-row block
                for qb in range(CH // P):
                    blk = ch * (CH // P) + qb
                    po = ops.tile([P, D], F32, tag="po", name="po")
                    for pb in range(KB):
                        for n0, nn in _nsplit(D):
                            nc.tensor.matmul(
                                po[:, n0 : n0 + nn],
                                lhsT=ctx[:, pb, qb * P : (qb + 1) * P],
                                rhs=wo_t[pb][:, n0 : n0 + nn],
                                start=(pb == 0),
                                stop=(pb == KB - 1),
                            )
                    t = x1_t[:, blk, :]
                    nc.vector.tensor_add(out=t, in0=po, in1=xr_t[:, blk, :])
                    _layernorm(nc, t, g1_t, bl1_t, eps_t, spool)

    # ---------------- Phase 3: FFN + LN2 ----------------
    with (
        tc.tile_pool(name="ffn_xt", bufs=1) as xtpool2,
        tc.tile_pool(name="ffn_h", bufs=1) as hpool,
        tc.tile_pool(name="ffn_w", bufs=1) as wfpool,
        tc.tile_pool(name="ffn_y", bufs=1) as ypool,
        tc.tile_pool(name="ffn_o", bufs=3) as opool,
        tc.tile_pool(name="ffn_tps", bufs=2, space="PSUM") as tps2,
        tc.tile_pool(name="ffn_h_ps", bufs=2, space="PSUM") as hps,
        tc.tile_pool(name="ffn_y_ps", bufs=2, space="PSUM") as yps,
        tc.tile_pool(name="ln_stats2", bufs=3) as spool2,
    ):
        x1T = xtpool2.tile([P, KB, QR], F32R)
        for blk in range(QR // P):
            for k in range(KB):
                tp = tps2.tile([P, P], F32R, tag="tps2", name="tps2")
                nc.tensor.transpose(tp, x1_t[:, blk, k * P : (k + 1) * P], ident)
                nc.vector.tensor_copy(out=x1T[:, k, blk * P : (blk + 1) * P], in_=tp)
        y_acc = ypool.tile([P, QR // P, D], F32, tag="y_acc")
        NH = 2  # ff halves
        FH = DFF // NH
        for half in range(NH):
            w1_t = [
                wfpool.tile([P, FH], F32R, tag=f"w1_{k}", name=f"w1_{k}")
                for k in range(KB)
            ]
            for k in range(KB):
                nc.sync.dma_start(
                    out=w1_t[k],
                    in_=w1[k * P : (k + 1) * P, half * FH : (half + 1) * FH],
                )
            h_t = hpool.tile([P, FH // P, QR], F32R, tag="h", name="h")
            for m in range(FH // P):
                mg = half * (FH // P) + m
                for n0, nn in _nsplit(QR):
                    ph = hps.tile([P, 512], F32, tag="ph", name="ph")
                    for k in range(KB):
                        nc.tensor.matmul(
                            ph[:, :nn],
                            lhsT=w1_t[k][:, m * P : (m + 1) * P],
                            rhs=x1T[:, k, n0 : n0 + nn],
                            start=(k == 0),
                            stop=(k == KB - 1),
                        )
                    nc.scalar.activation(
                        out=h_t[:, m, n0 : n0 + nn],
                        in_=ph[:, :nn],
                        func=AF.Relu,
                        bias=b1_t[:, mg : mg + 1],
                        scale=1.0,
                    )
            w2_t = [
                wfpool.tile([P, D], F32R, tag=f"w2_{k}", name=f"w2_{k}")
                for k in range(FH // P)
            ]
            for k in range(FH // P):
                nc.sync.dma_start(
                    out=w2_t[k],
                    in_=w2[half * FH + k * P : half * FH + (k + 1) * P, :],
                )
            for blk in range(QR // P):
                py = yps.tile([P, D], F32, tag="py", name="py")
                for k in range(FH // P):
                    for n0, nn in _nsplit(D):
                        nc.tensor.matmul(
                            py[:, n0 : n0 + nn],
                            lhsT=h_t[:, k, blk * P : (blk + 1) * P],
                            rhs=w2_t[k][:, n0 : n0 + nn],
                            start=(k == 0),
                            stop=(k == FH // P - 1),
                        )
                if half == 0:
                    nc.vector.tensor_copy(out=y_acc[:, blk, :], in_=py)
                else:
                    t = opool.tile([P, D], F32, tag="obuf", name="obuf")
                    nc.vector.tensor_add(out=t, in0=py, in1=y_acc[:, blk, :])
                    nc.vector.tensor_add(out=t, in0=t, in1=b2_t)
                    nc.vector.tensor_add(out=t, in0=t, in1=x1_t[:, blk, :])
                    _layernorm(nc, t, g2_t, bl2_t, eps_t, spool2)
                    nc.sync.dma_start(out=out[blk * P : (blk + 1) * P, :], in_=t)
    es.close()


# ---------------- host side ----------------


def _numpy_reference(x, mask, Wq, bq, Wk, bk, Wv, bv, Wo, bo, W1, b1, W2, b2,
                     ln1_g, ln1_b, ln2_g, ln2_b):
    def ln(t, g, b, eps=1e-5):
        mu = t.mean(-1, keepdims=True)
        var = t.var(-1, keepdims=True)
        return (t - mu) / np.sqrt(var + eps) * g + b

    b_, s_, d_ = x.shape
    dk = d_ // H

    def split(h):
        return h.reshape(b_, s_, H, dk).transpose(0, 2, 1, 3)

    Q = split(x @ Wq + bq)
    K = split(x @ Wk + bk)
    V = split(x @ Wv + bv)
    sc = np.einsum("bhqd,bhkd->bhqk", Q, K) / np.sqrt(dk)
    sc = np.where(mask == 0, np.float32(-1e9), sc)
    sc = sc - sc.max(-1, keepdims=True)
    p = np.exp(sc)
    p = p / p.sum(-1, keepdims=True)
    ctx = np.einsum("bhqk,bhkd->bhqd", p, V)
    ctx = ctx.transpose(0, 2, 1, 3).reshape(b_, s_, d_)
    x1 = ln(x + ctx @ Wo + bo, ln1_g, ln1_b)
    y = np.maximum(x1 @ W1 + b1, 0.0) @ W2 + b2
    return ln(x1 + y, ln2_g, ln2_b).astype(np.float32)


def _get_program():
    global _PROGRAM
    if _PROGRAM is None:
        _PROGRAM = build_program()
    return _PROGRAM


def _core_rows(c):
    j = c % 4
    return c // 4, np.r_[j * CH : (j + 1) * CH, (7 - j) * CH : (8 - j) * CH]


def _make_in_maps(inputs):
    x = np.asarray(inputs["x"], dtype=np.float32)
    m2 = np.asarray(inputs["mask"]).reshape(S, S)
    maskf = (m2 != 0).astype(np.float32)
    f32 = lambda k: np.ascontiguousarray(np.asarray(inputs[k], dtype=np.float32))
    xr_const = (f32("bv") @ f32("Wo") + f32("bo")).astype(np.float32)

    common = {
        "ident": np.eye(P, dtype=np.float32),
        "ones64": np.ones(DK, dtype=np.float32),
        "vones": np.ones(H, dtype=np.float32),
        "wq": f32("Wq"), "wk": f32("Wk"), "wv": f32("Wv"), "wo": f32("Wo"),
        "w1": f32("W1"), "w2": f32("W2"),
        "bq": f32("bq"), "bk": f32("bk"), "b1": f32("b1"), "b2": f32("b2"),
        "g1": f32("ln1_g"), "bl1": f32("ln1_b"),
        "g2": f32("ln2_g"), "bl2": f32("ln2_b"),
    }
    in_maps = []
    for c in range(NCORES):
        b, rows = _core_rows(c)
        xq_c = np.ascontiguousarray(x[b][rows])
        bias_a = ((1.0 - maskf[rows[:CH], : EXT[0]]) * NEG).T
        bias_b = ((1.0 - maskf[rows[CH:], : EXT[1]]) * NEG).T
        in_maps.append(
            {
                "xb": np.ascontiguousarray(x[b]),
                "xq": xq_c,
                "xr": (xq_c + xr_const).astype(np.float32),
                "biasA": np.ascontiguousarray(bias_a, dtype=np.float32),
                "biasB": np.ascontiguousarray(bias_b, dtype=np.float32),
                **common,
            }
        )
    return in_maps


def kernel(x, mask, Wq, bq, Wk, bk, Wv, bv, Wo, bo, W1, b1, W2, b2,
           ln1_g, ln1_b, ln2_g, ln2_b):
    inputs = dict(x=x, mask=mask, Wq=Wq, bq=bq, Wk=Wk, bk=bk, Wv=Wv, bv=bv,
                  Wo=Wo, bo=bo, W1=W1, b1=b1, W2=W2, b2=b2, ln1_g=ln1_g,
                  ln1_b=ln1_b, ln2_g=ln2_g, ln2_b=ln2_b)
    x = np.asarray(x, dtype=np.float32)
    m2 = np.asarray(mask).reshape(-1)
    ok = x.shape == (B, S, D) and np.asarray(mask).size == S * S and np.array_equal(
        np.asarray(mask).reshape(S, S) != 0,
        np.tril(np.ones((S, S), dtype=bool)),
    )
    if not ok:
        args = [np.asarray(a, dtype=np.float32) for a in
                (Wq, bq, Wk, bk, Wv, bv, Wo, bo, W1, b1, W2, b2,
                 ln1_g, ln1_b, ln2_g, ln2_b)]
        return _numpy_reference(x, np.asarray(mask), *args)

    nc = _get_program()
    in_maps = _make_in_maps(inputs)
    res = run_bass_kernel_spmd(nc, in_maps, list(range(NCORES)))
    outp = np.empty((B, S, D), dtype=np.float32)
    for c in range(NCORES):
        b, rows = _core_rows(c)
        outp[b][rows] = res.results[c]["out"]
    return outp


# revision 15
# speedup vs baseline: 9.3654x; 9.3654x over previous
"""Trainium2 Bass kernel for a causal transformer decoder block.

Sharding: sequence-parallel, no collectives. 8 cores = 2 batches x 4
query-chunk groups. Core c (j = c % 4, b = c // 4) handles batch b and two
256-row query chunks: rows [256j, 256j+256) and [256(7-j), 256(8-j)).
Chunk A runs attention against keys [0, 1024), chunk B against [0, 2048)
(padded to a uniform extent so all 8 cores execute one SPMD program; the
causal mask is applied via host-precomputed additive bias tiles, so the
padding is data, not control flow). Every core computes full-sequence K/V
for its batch; weights are replicated. Matmuls run in float32r (full-rate
fp32 PE streaming mode) with fp32 PSUM accumulation.
"""

import os
import sys

for _p in ("/opt/trn_rl_repo", "/root/.axon_site/_ro/trn_rl_repo"):
    if os.path.isdir(_p) and _p not in sys.path:
        sys.path.insert(0, _p)

import numpy as np

import concourse.bacc as bacc
import concourse.bass as bass
import concourse.mybir as mybir
import concourse.tile as tile
from concourse.bass_utils import run_bass_kernel_spmd

P = 128
B, S, D = 2, 2048, 768
H, DK = 12, 64
DFF = 3072
NCORES = 8
CH = 256  # query chunk rows per chunk (2 chunks per core)
QR = 2 * CH  # query rows per core
EXT = (1024, 2048)  # padded key extents for chunk A / chunk B
KB = D // P  # 6 contraction blocks over D
MB = DFF // P  # 24 blocks over DFF
NEG = -8.0e9  # raw-score mask offset; * 0.125 -> -1e9 before exp

F32 = mybir.dt.float32
F32R = mybir.dt.float32r
AF = mybir.ActivationFunctionType

_PROGRAM = None


def _bcast_row(nc, dram_ap, n):
    """AP view of a [n] DRAM vector broadcast to [P, n]."""
    return bass.AP(tensor=dram_ap.tensor, offset=dram_ap.offset, ap=[[0, P], [1, n]])


def _nsplit(n):
    """Split a free dim into <=512 chunks (each >=256 for f32r full rate)."""
    out, s = [], 0
    while n - s > 512:
        out.append((s, 512))
        s += 512
    out.append((s, n - s))
    return out


def build_program_for_sim():
    return build_program(finalize=False)


def build_program(finalize=True, n_reps=1):
    nc = bacc.Bacc(None)

    xb = nc.declare_dram_parameter("xb", [S, D], F32R, isOutput=False)
    xq = nc.declare_dram_parameter("xq", [QR, D], F32R, isOutput=False)
    xr = nc.declare_dram_parameter("xr", [QR, D], F32, isOutput=False)
    wq = nc.declare_dram_parameter("wq", [D, D], F32R, isOutput=False)
    wk = nc.declare_dram_parameter("wk", [D, D], F32R, isOutput=False)
    wv = nc.declare_dram_parameter("wv", [D, D], F32R, isOutput=False)
    wo = nc.declare_dram_parameter("wo", [D, D], F32R, isOutput=False)
    w1 = nc.declare_dram_parameter("w1", [D, DFF], F32R, isOutput=False)
    w2 = nc.declare_dram_parameter("w2", [DFF, D], F32R, isOutput=False)
    bq = nc.declare_dram_parameter("bq", [D], F32, isOutput=False)
    bk = nc.declare_dram_parameter("bk", [D], F32, isOutput=False)
    b1 = nc.declare_dram_parameter("b1", [DFF], F32, isOutput=False)
    b2 = nc.declare_dram_parameter("b2", [D], F32, isOutput=False)
    g1 = nc.declare_dram_parameter("g1", [D], F32, isOutput=False)
    bl1 = nc.declare_dram_parameter("bl1", [D], F32, isOutput=False)
    g2 = nc.declare_dram_parameter("g2", [D], F32, isOutput=False)
    bl2 = nc.declare_dram_parameter("bl2", [D], F32, isOutput=False)
    ident_d = nc.declare_dram_parameter("ident", [P, P], F32R, isOutput=False)
    ones_d = nc.declare_dram_parameter("ones64", [DK], F32R, isOutput=False)
    vones_d = nc.declare_dram_parameter("vones", [H], F32R, isOutput=False)
    biasA = nc.declare_dram_parameter("biasA", [EXT[0], CH], F32, isOutput=False)
    biasB = nc.declare_dram_parameter("biasB", [EXT[1], CH], F32, isOutput=False)
    out = nc.declare_dram_parameter("out", [QR, D], F32, isOutput=True)

    io = locals()
    with tile.TileContext(nc) as tc:
        for _ in range(n_reps):
            _emit(nc, tc, io)
    if finalize:
        if not nc.is_finalized():
            nc.finalize()
    else:
        nc.compile()
    return nc


def _layernorm(nc, t, gb, bb, eps_t, stats_pool):
    """In-place LayerNorm of SBUF tile t [P, D] over the free dim."""
    sub = 256  # gcd(512, 768)
    nsub = D // sub
    stats = stats_pool.tile([P, nsub, 6], F32, tag="ln_stats")
    tv = t.rearrange("p (n s) -> p n s", s=sub)
    for i in range(nsub):
        nc.vector.bn_stats(out=stats[:, i, :], in_=tv[:, i, :])
    mv = stats_pool.tile([P, 2], F32, tag="ln_mv")
    nc.vector.bn_aggr(out=mv, in_=stats)
    # mv[:,1] = 1/sqrt(var + eps)
    nc.scalar.activation(
        out=mv[:, 1:2], in_=mv[:, 1:2], func=AF.Sqrt, bias=eps_t, scale=1.0
    )
    nc.vector.reciprocal(out=mv[:, 1:2], in_=mv[:, 1:2])
    nc.vector.tensor_scalar(
        out=t,
        in0=t,
        scalar1=mv[:, 0:1],
        scalar2=mv[:, 1:2],
        op0=mybir.AluOpType.subtract,
        op1=mybir.AluOpType.mult,
    )
    nc.vector.tensor_mul(out=t, in0=t, in1=gb)
    nc.vector.tensor_add(out=t, in0=t, in1=bb)


def _emit(nc, tc, io):
    xb, xq, xr = io["xb"], io["xq"], io["xr"]
    ident_d, ones_d, vones_d = io["ident_d"], io["ones_d"], io["vones_d"]
    wq, wk, wv, wo, w1, w2 = (io[k] for k in ("wq", "wk", "wv", "wo", "w1", "w2"))
    bq, bk, b1, b2 = io["bq"], io["bk"], io["b1"], io["b2"]
    g1, bl1, g2, bl2 = io["g1"], io["bl1"], io["g2"], io["bl2"]
    biasA, biasB, out = io["biasA"], io["biasB"], io["out"]

    from contextlib import ExitStack

    es = ExitStack()
    const = es.enter_context(tc.tile_pool(name="const", bufs=1))
    ident = const.tile([P, P], F32R)
    nc.sync.dma_start(out=ident, in_=ident_d[:, :])
    eps_t = const.tile([P, 1], F32)
    nc.vector.memset(eps_t, 1e-5)
    ones_t = const.tile([P, DK], F32R)
    nc.sync.dma_start(out=ones_t, in_=_bcast_row(nc, ones_d[:], DK))
    vones_t = const.tile([P, H], F32R)
    nc.sync.dma_start(out=vones_t, in_=_bcast_row(nc, vones_d[:], H))
    bq_t = const.tile([P, KB], F32)
    nc.gpsimd.dma_start(out=bq_t, in_=bq[:].rearrange("(k p) -> p k", p=P))
    bk_t = const.tile([P, KB], F32)
    nc.gpsimd.dma_start(out=bk_t, in_=bk[:].rearrange("(k p) -> p k", p=P))
    b1_t = const.tile([P, MB], F32)
    nc.gpsimd.dma_start(out=b1_t, in_=b1[:].rearrange("(m p) -> p m", p=P))
    g1_t = const.tile([P, D], F32)
    nc.sync.dma_start(out=g1_t, in_=_bcast_row(nc, g1[:], D))
    bl1_t = const.tile([P, D], F32)
    nc.sync.dma_start(out=bl1_t, in_=_bcast_row(nc, bl1[:], D))
    g2_t = const.tile([P, D], F32)
    nc.sync.dma_start(out=g2_t, in_=_bcast_row(nc, g2[:], D))
    bl2_t = const.tile([P, D], F32)
    nc.sync.dma_start(out=bl2_t, in_=_bcast_row(nc, bl2[:], D))
    b2_t = const.tile([P, D], F32)
    nc.sync.dma_start(out=b2_t, in_=_bcast_row(nc, b2[:], D))
    xr_t = const.tile([P, QR // P, D], F32)
    nc.sync.dma_start(out=xr_t, in_=xr[:, :].rearrange("(qb p) d -> p qb d", p=P))
    x1_t = const.tile([P, QR // P, D], F32R)  # post-LN1 activations

    with (
        tc.tile_pool(name="persist", bufs=1) as persist,
    ):
        kT = persist.tile([P, KB, S], F32R)  # K^T: [d within pb, pb, key]
        qT = persist.tile([P, KB, QR], F32R)  # Q^T
        # V with a ones column per head: [k within block, key block, h*65+e]
        vA = persist.tile([P, S // P, H * (DK + 1)], F32R)
        vA4 = vA.rearrange("p s (h e) -> p s h e", e=DK + 1)
        for s in range(S // P):
            nc.vector.tensor_copy(out=vA4[:, s, :, DK], in_=vones_t)

        # ---------------- Phase 1: QKV projections ----------------
        with (
            tc.tile_pool(name="qkv_x", bufs=3) as xpool,
            tc.tile_pool(name="qkv_ps", bufs=2, space="PSUM") as pps,
            tc.tile_pool(name="qkv_tps", bufs=2, space="PSUM") as tps,
        ):

            def transpose_chunk(src_dram, row0, xt_tile, col0, ncols):
                for sb in range(ncols // P):
                    xtile = xpool.tile([P, D], F32R, tag="xload", name="xload")
                    nc.sync.dma_start(
                        out=xtile,
                        in_=src_dram[row0 + sb * P : row0 + (sb + 1) * P, :],
                    )
                    for k in range(KB):
                        tp = tps.tile([P, P], F32R, tag="tps", name="tps")
                        nc.tensor.transpose(tp, xtile[:, k * P : (k + 1) * P], ident)
                        nc.vector.tensor_copy(
                            out=xt_tile[:, k, col0 + sb * P : col0 + (sb + 1) * P],
                            in_=tp,
                        )

            # Q^T from xq
            with (
                tc.tile_pool(name="q_w", bufs=1) as wqpool,
                tc.tile_pool(name="q_xt", bufs=1) as xqtpool,
            ):
                wq_t = [
                    wqpool.tile([P, D], F32R, tag=f"wq{k}", name=f"wq{k}")
                    for k in range(KB)
                ]
                for k in range(KB):
                    nc.sync.dma_start(out=wq_t[k], in_=wq[k * P : (k + 1) * P, :])
                xqT = xqtpool.tile([P, KB, QR], F32R, tag="xqT", name="xqT")
                transpose_chunk(xq, 0, xqT, 0, QR)
                for pb in range(KB):
                    for n0, nn in _nsplit(QR):
                        ps = pps.tile([P, 512], F32, tag="proj", name="proj")
                        for k in range(KB):
                            nc.tensor.matmul(
                                ps[:, :nn],
                                lhsT=wq_t[k][:, pb * P : (pb + 1) * P],
                                rhs=xqT[:, k, n0 : n0 + nn],
                                start=(k == 0),
                                stop=(k == KB - 1),
                            )
                        nc.vector.tensor_scalar_add(
                            out=qT[:, pb, n0 : n0 + nn],
                            in0=ps[:, :nn],
                            scalar1=bq_t[:, pb : pb + 1],
                        )

            # K^T and V per 512-row key chunk
            with (
                tc.tile_pool(name="kv_w", bufs=1) as wkvpool,
                tc.tile_pool(name="kv_xt", bufs=1) as xbtpool,
            ):
                wk_t = [
                    wkvpool.tile([P, D], F32R, tag=f"wk{k}", name=f"wk{k}")
                    for k in range(KB)
                ]
                wv_t = [
                    wkvpool.tile([P, D], F32R, tag=f"wv{k}", name=f"wv{k}")
                    for k in range(KB)
                ]
                for k in range(KB):
                    nc.sync.dma_start(out=wk_t[k], in_=wk[k * P : (k + 1) * P, :])
                    nc.sync.dma_start(out=wv_t[k], in_=wv[k * P : (k + 1) * P, :])
                for sc in range(S // 512):
                    xbT = xbtpool.tile([P, KB, 512], F32R, tag="xbT", name="xbT")
                    transpose_chunk(xb, sc * 512, xbT, 0, 512)
                    for pb in range(KB):
                        ps = pps.tile([P, 512], F32, tag="proj", name="proj")
                        for k in range(KB):
                            nc.tensor.matmul(
                                ps,
                                lhsT=wk_t[k][:, pb * P : (pb + 1) * P],
                                rhs=xbT[:, k, :],
                                start=(k == 0),
                                stop=(k == KB - 1),
                            )
                        nc.vector.tensor_scalar_add(
                            out=kT[:, pb, sc * 512 : (sc + 1) * 512],
                            in0=ps,
                            scalar1=bk_t[:, pb : pb + 1],
                        )
                    for sb in range(4):
                        ps = pps.tile([P, D], F32, tag="vproj", name="vproj")
                        for k in range(KB):
                            for n0, nn in _nsplit(D):
                                nc.tensor.matmul(
                                    ps[:, n0 : n0 + nn],
                                    lhsT=xbT[:, k, sb * P : (sb + 1) * P],
                                    rhs=wv_t[k][:, n0 : n0 + nn],
                                    start=(k == 0),
                                    stop=(k == KB - 1),
                                )
                        nc.vector.tensor_copy(
                            out=vA4[:, sc * 4 + sb, :, 0:DK],
                            in_=ps.rearrange("p (h d) -> p h d", d=DK),
                        )

        # ---------------- Phase 2: attention ----------------
        with (
            tc.tile_pool(name="att_wo", bufs=1) as wopool,
            tc.tile_pool(name="att_bias", bufs=1) as bpool,
            tc.tile_pool(name="att_p", bufs=3) as ppool,
            tc.tile_pool(name="att_ctx", bufs=2) as cpool,
            tc.tile_pool(name="att_dn", bufs=2) as dpool,
            tc.tile_pool(name="att_st", bufs=2, space="PSUM") as stps,
            tc.tile_pool(name="att_cx", bufs=2, space="PSUM") as cxps,
            tc.tile_pool(name="att_bc", bufs=1, space="PSUM") as bcps,
            tc.tile_pool(name="att_o", bufs=1, space="PSUM") as ops,
            tc.tile_pool(name="ln_stats", bufs=3) as spool,
        ):
            wo_t = [
                wopool.tile([P, D], F32R, tag=f"wo{k}", name=f"wo{k}")
                for k in range(KB)
            ]
            for k in range(KB):
                nc.sync.dma_start(out=wo_t[k], in_=wo[k * P : (k + 1) * P, :])

            for ch, (bias_d, ext) in enumerate(zip((biasA, biasB), EXT)):
                nkb = ext // P
                bias_t = bpool.tile(
                    [P, EXT[1] // P, CH], F32, tag="bias", name="bias"
                )[:, :nkb, :]
                nc.sync.dma_start(
                    out=bias_t,
                    in_=bias_d[:, :].rearrange("(n p) q -> p n q", p=P),
                )
                ctx = cpool.tile([P, KB, CH], F32R, tag="ctx", name="ctx")
                for h in range(H):
                    pb, base = h // 2, DK * (h % 2)
                    cx = cxps.tile([DK + 1, CH], F32, tag="cx", name="cx")
                    for kb in range(nkb):
                        st = stps.tile([P, CH], F32, tag="st", name="st")
                        nc.tensor.matmul(
                            st,
                            lhsT=kT[base : base + DK, pb, kb * P : (kb + 1) * P],
                            rhs=qT[base : base + DK, pb, ch * CH : (ch + 1) * CH],
                            start=True,
                            stop=True,
                        )
                        nc.vector.tensor_add(out=st, in0=st, in1=bias_t[:, kb, :])
                        pt = ppool.tile([P, CH], F32R, tag="pt", name="pt")
                        nc.scalar.activation(out=pt, in_=st, func=AF.Exp, scale=0.125)
                        nc.tensor.matmul(
                            cx,
                            lhsT=vA[:, kb, h * (DK + 1) : (h + 1) * (DK + 1)],
                            rhs=pt,
                            start=(kb == 0),
                            stop=(kb == nkb - 1),
                        )
                    # denominator row -> reciprocal broadcast -> normalized ctx^T
                    dn = dpool.tile([P, CH], F32R, tag="dn", name="dn")
                    nc.vector.tensor_copy(
                        out=dn[DK : DK + 1, :], in_=cx[DK : DK + 1, :]
                    )
                    bc = bcps.tile([DK, CH], F32, tag="bc", name="bc")
                    nc.tensor.matmul(
                        bc,
                        lhsT=ones_t[DK : DK + 1, :],
                        rhs=dn[DK : DK + 1, :],
                        start=True,
                        stop=True,
                    )
                    rc = dpool.tile([DK, CH], F32, tag="rc", name="rc")
                    nc.vector.reciprocal(out=rc, in_=bc)
                    nc.vector.tensor_mul(
                        out=ctx[base : base + DK, pb, :], in0=cx[0:DK, :], in1=rc
                    )
                # O-projection + residual + LN1 per 128-row block
                for qb in range(CH // P):
                    blk = ch * (CH // P) + qb
                    po = ops.tile([P, D], F32, tag="po", name="po")
                    for pb in range(KB):
                        for n0, nn in _nsplit(D):
                            nc.tensor.matmul(
                                po[:, n0 : n0 + nn],
                                lhsT=ctx[:, pb, qb * P : (qb + 1) * P],
                                rhs=wo_t[pb][:, n0 : n0 + nn],
                                start=(pb == 0),
                                stop=(pb == KB - 1),
                            )
                    t = x1_t[:, blk, :]
                    nc.vector.tensor_add(out=t, in0=po, in1=xr_t[:, blk, :])
                    _layernorm(nc, t, g1_t, bl1_t, eps_t, spool)

    # ---------------- Phase 3: FFN + LN2 ----------------
    with (
        tc.tile_pool(name="ffn_xt", bufs=1) as xtpool2,
        tc.tile_pool(name="ffn_h", bufs=1) as hpool,
        tc.tile_pool(name="ffn_w", bufs=1) as wfpool,
        tc.tile_pool(name="ffn_y", bufs=1) as ypool,
        tc.tile_pool(name="ffn_o", bufs=3) as opool,
        tc.tile_pool(name="ffn_tps", bufs=2, space="PSUM") as tps2,
        tc.tile_pool(name="ffn_h_ps", bufs=2, space="PSUM") as hps,
        tc.tile_pool(name="ffn_y_ps", bufs=2, space="PSUM") as yps,
        tc.tile_pool(name="ln_stats2", bufs=3) as spool2,
    ):
        x1T = xtpool2.tile([P, KB, QR], F32R)
        for blk in range(QR // P):
            for k in range(KB):
                tp = tps2.tile([P, P], F32R, tag="tps2", name="tps2")
                nc.tensor.transpose(tp, x1_t[:, blk, k * P : (k + 1) * P], ident)
                nc.vector.tensor_copy(out=x1T[:, k, blk * P : (blk + 1) * P], in_=tp)
        y_acc = ypool.tile([P, QR // P, D], F32, tag="y_acc")
        NH = 2  # ff halves
        FH = DFF // NH
        for half in range(NH):
            w1_t = [
                wfpool.tile([P, FH], F32R, tag=f"w1_{k}", name=f"w1_{k}")
                for k in range(KB)
            ]
            for k in range(KB):
                nc.sync.dma_start(
                    out=w1_t[k],
                    in_=w1[k * P : (k + 1) * P, half * FH : (half + 1) * FH],
                )
            h_t = hpool.tile([P, FH // P, QR], F32R, tag="h", name="h")
            for m in range(FH // P):
                mg = half * (FH // P) + m
                for n0, nn in _nsplit(QR):
                    ph = hps.tile([P, 512], F32, tag="ph", name="ph")
                    for k in range(KB):
                        nc.tensor.matmul(
                            ph[:, :nn],
                            lhsT=w1_t[k][:, m * P : (m + 1) * P],
                            rhs=x1T[:, k, n0 : n0 + nn],
                            start=(k == 0),
                            stop=(k == KB - 1),
                        )
                    nc.scalar.activation(
                        out=h_t[:, m, n0 : n0 + nn],
                        in_=ph[:, :nn],
                        func=AF.Relu,
                        bias=b1_t[:, mg : mg + 1],
                        scale=1.0,
                    )
            w2_t = [
                wfpool.tile([P, D], F32R, tag=f"w2_{k}", name=f"w2_{k}")
                for k in range(FH // P)
            ]
            for k in range(FH // P):
                nc.sync.dma_start(
                    out=w2_t[k],
                    in_=w2[half * FH + k * P : half * FH + (k + 1) * P, :],
                )
            for blk in range(QR // P):
                py = yps.tile([P, D], F32, tag="py", name="py")
                for k in range(FH // P):
                    for n0, nn in _nsplit(D):
                        nc.tensor.matmul(
                            py[:, n0 : n0 + nn],
                            lhsT=h_t[:, k, blk * P : (blk + 1) * P],
                            rhs=w2_t[k][:, n0 : n0 + nn],
                            start=(k == 0),
                            stop=(k == FH // P - 1),
                        )
                if half == 0:
                    nc.vector.tensor_copy(out=y_acc[:, blk, :], in_=py)
                else:
                    t = opool.tile([P, D], F32, tag="obuf", name="obuf")
                    nc.vector.tensor_add(out=t, in0=py, in1=y_acc[:, blk, :])
                    nc.vector.tensor_add(out=t, in0=t, in1=b2_t)
                    nc.vector.tensor_add(out=t, in0=t, in1=x1_t[:, blk, :])
                    _layernorm(nc, t, g2_t, bl2_t, eps_t, spool2)
                    nc.sync.dma_start(out=out[blk * P : (blk + 1) * P, :], in_=t)
    es.close()


# ---------------- host side ----------------


def _numpy_reference(x, mask, Wq, bq, Wk, bk, Wv, bv, Wo, bo, W1, b1, W2, b2,
                     ln1_g, ln1_b, ln2_g, ln2_b):
    def ln(t, g, b, eps=1e-5):
        mu = t.mean(-1, keepdims=True)
        var = t.var(-1, keepdims=True)
        return (t - mu) / np.sqrt(var + eps) * g + b

    b_, s_, d_ = x.shape
    dk = d_ // H

    def split(h):
        return h.reshape(b_, s_, H, dk).transpose(0, 2, 1, 3)

    Q = split(x @ Wq + bq)
    K = split(x @ Wk + bk)
    V = split(x @ Wv + bv)
    sc = np.einsum("bhqd,bhkd->bhqk", Q, K) / np.sqrt(dk)
    sc = np.where(mask == 0, np.float32(-1e9), sc)
    sc = sc - sc.max(-1, keepdims=True)
    p = np.exp(sc)
    p = p / p.sum(-1, keepdims=True)
    ctx = np.einsum("bhqk,bhkd->bhqd", p, V)
    ctx = ctx.transpose(0, 2, 1, 3).reshape(b_, s_, d_)
    x1 = ln(x + ctx @ Wo + bo, ln1_g, ln1_b)
    y = np.maximum(x1 @ W1 + b1, 0.0) @ W2 + b2
    return ln(x1 + y, ln2_g, ln2_b).astype(np.float32)


def _get_program():
    global _PROGRAM
    if _PROGRAM is None:
        _PROGRAM = build_program()
    return _PROGRAM


def _core_rows(c):
    j = c % 4
    return c // 4, np.r_[j * CH : (j + 1) * CH, (7 - j) * CH : (8 - j) * CH]


def _make_in_maps(inputs):
    x = np.asarray(inputs["x"], dtype=np.float32)
    m2 = np.asarray(inputs["mask"]).reshape(S, S)
    maskf = (m2 != 0).astype(np.float32)
    f32 = lambda k: np.ascontiguousarray(np.asarray(inputs[k], dtype=np.float32))
    xr_const = (f32("bv") @ f32("Wo") + f32("bo")).astype(np.float32)

    common = {
        "ident": np.eye(P, dtype=np.float32),
        "ones64": np.ones(DK, dtype=np.float32),
        "vones": np.ones(H, dtype=np.float32),
        "wq": f32("Wq"), "wk": f32("Wk"), "wv": f32("Wv"), "wo": f32("Wo"),
        "w1": f32("W1"), "w2": f32("W2"),
        "bq": f32("bq"), "bk": f32("bk"), "b1": f32("b1"), "b2": f32("b2"),
        "g1": f32("ln1_g"), "bl1": f32("ln1_b"),
        "g2": f32("ln2_g"), "bl2": f32("ln2_b"),
    }
    in_maps = []
    for c in range(NCORES):
        b, rows = _core_rows(c)
        xq_c = np.ascontiguousarray(x[b][rows])
        bias_a = ((1.0 - maskf[rows[:CH], : EXT[0]]) * NEG).T
        bias_b = ((1.0 - maskf[rows[CH:], : EXT[1]]) * NEG).T
        in_maps.append(
            {
                "xb": np.ascontiguousarray(x[b]),
                "xq": xq_c,
                "xr": (xq_c + xr_const).astype(np.float32),
                "biasA": np.ascontiguousarray(bias_a, dtype=np.float32),
                "biasB": np.ascontiguousarray(bias_b, dtype=np.float32),
                **common,
            }
        )
    return in_maps


def kernel(x, mask, Wq, bq, Wk, bk, Wv, bv, Wo, bo, W1, b1, W2, b2,
           ln1_g, ln1_b, ln2_g, ln2_b):
    inputs = dict(x=x, mask=mask, Wq=Wq, bq=bq, Wk=Wk, bk=bk, Wv=Wv, bv=bv,
                  Wo=Wo, bo=bo, W1=W1, b1=b1, W2=W2, b2=b2, ln1_g=ln1_g,
                  ln1_b=ln1_b, ln2_g=ln2_g, ln2_b=ln2_b)
    x = np.asarray(x, dtype=np.float32)
    m2 = np.asarray(mask).reshape(-1)
    ok = x.shape == (B, S, D) and np.asarray(mask).size == S * S and np.array_equal(
        np.asarray(mask).reshape(S, S) != 0,
        np.tril(np.ones((S, S), dtype=bool)),
    )
    if not ok:
        args = [np.asarray(a, dtype=np.float32) for a in
                (Wq, bq, Wk, bk, Wv, bv, Wo, bo, W1, b1, W2, b2,
                 ln1_g, ln1_b, ln2_g, ln2_b)]
        return _numpy_reference(x, np.asarray(mask), *args)

    nc = _get_program()
    in_maps = _make_in_maps(inputs)
    res = run_bass_kernel_spmd(nc, in_maps, list(range(NCORES)))
    outp = np.empty((B, S, D), dtype=np.float32)
    for c in range(NCORES):
        b, rows = _core_rows(c)
        outp[b][rows] = res.results[c]["out"]
    return outp


# revision 22
# speedup vs baseline: 273.1660x; 29.1674x over previous
"""Trainium2 Bass kernel for a causal transformer decoder block.

Sharding: sequence-parallel, no collectives. 8 cores = 2 batches x 4
query-chunk groups. Core c (j = c % 4, b = c // 4) handles batch b and two
256-row query chunks: rows [256j, 256j+256) and [256(7-j), 256(8-j)).
Chunk A runs attention against keys [0, 1024), chunk B against [0, 2048)
(padded to a uniform extent so all 8 cores execute one SPMD program; the
causal mask is applied via host-precomputed additive bias tiles, so the
padding is data, not control flow). Every core computes full-sequence K/V
for its batch; weights are replicated. Matmuls run in float32r (full-rate
fp32 PE streaming mode) with fp32 PSUM accumulation.
"""

import os
import sys

for _p in ("/opt/trn_rl_repo", "/root/.axon_site/_ro/trn_rl_repo"):
    if os.path.isdir(_p) and _p not in sys.path:
        sys.path.insert(0, _p)

import numpy as np

import concourse.bacc as bacc
import concourse.bass as bass
import concourse.mybir as mybir
import concourse.tile as tile
from concourse.bass_utils import run_bass_kernel_spmd

P = 128
B, S, D = 2, 2048, 768
H, DK = 12, 64
DFF = 3072
NCORES = 8
CH = 256  # query chunk rows per chunk (2 chunks per core)
QR = 2 * CH  # query rows per core
EXT = (1280, 2048)  # padded key-slot extents (reordered keys)
KB = D // P  # 6 contraction blocks over D
MB = DFF // P  # 24 blocks over DFF
NEG = -8.0e9  # raw-score mask offset; * 0.125 -> -1e9 before exp

F32 = mybir.dt.float32
F32R = mybir.dt.float32r
AF = mybir.ActivationFunctionType

_PROGRAM = None


def _bcast_row(nc, dram_ap, n):
    """AP view of a [n] DRAM vector broadcast to [P, n]."""
    return bass.AP(tensor=dram_ap.tensor, offset=dram_ap.offset, ap=[[0, P], [1, n]])


def _nsplit(n):
    """Split a free dim into <=512 chunks (each >=256 for f32r full rate)."""
    out, s = [], 0
    while n - s > 512:
        out.append((s, 512))
        s += 512
    out.append((s, n - s))
    return out


def build_program_for_sim():
    return build_program(finalize=False)


def build_program(finalize=True, n_reps=1):
    nc = bacc.Bacc(None)

    xb = nc.declare_dram_parameter("xb", [S, D], F32R, isOutput=False)
    xq = nc.declare_dram_parameter("xq", [QR, D], F32R, isOutput=False)
    xr = nc.declare_dram_parameter("xr", [QR, D], F32, isOutput=False)
    wq = nc.declare_dram_parameter("wq", [D, D], F32R, isOutput=False)
    wk = nc.declare_dram_parameter("wk", [D, D], F32R, isOutput=False)
    wv = nc.declare_dram_parameter("wv", [D, D], F32R, isOutput=False)
    wo = nc.declare_dram_parameter("wo", [D, D], F32R, isOutput=False)
    w1 = nc.declare_dram_parameter("w1", [D, DFF], F32R, isOutput=False)
    w2 = nc.declare_dram_parameter("w2", [DFF, D], F32R, isOutput=False)
    bq = nc.declare_dram_parameter("bq", [D], F32, isOutput=False)
    bk = nc.declare_dram_parameter("bk", [D], F32, isOutput=False)
    b1 = nc.declare_dram_parameter("b1", [DFF], F32, isOutput=False)
    b2 = nc.declare_dram_parameter("b2", [D], F32, isOutput=False)
    g1 = nc.declare_dram_parameter("g1", [D], F32, isOutput=False)
    bl1 = nc.declare_dram_parameter("bl1", [D], F32, isOutput=False)
    g2 = nc.declare_dram_parameter("g2", [D], F32, isOutput=False)
    bl2 = nc.declare_dram_parameter("bl2", [D], F32, isOutput=False)
    ident_d = nc.declare_dram_parameter("ident", [P, P], F32R, isOutput=False)
    ones_d = nc.declare_dram_parameter("ones64", [DK], F32R, isOutput=False)
    vones_d = nc.declare_dram_parameter("vones", [H], F32R, isOutput=False)
    kbias = nc.declare_dram_parameter("kbias", [2, S], F32, isOutput=False)
    tril0 = nc.declare_dram_parameter("tril0", [P, CH], F32, isOutput=False)
    tril1 = nc.declare_dram_parameter("tril1", [P, CH], F32, isOutput=False)
    out = nc.declare_dram_parameter("out", [QR, D], F32, isOutput=True)

    io = locals()
    with tile.TileContext(nc) as tc:
        for _ in range(n_reps):
            _emit(nc, tc, io)
    if finalize:
        if not nc.is_finalized():
            nc.finalize()
    else:
        nc.compile()
    return nc


def _layernorm(nc, t, gb, bb, eps_t, stats_pool):
    """In-place LayerNorm of SBUF tile t [P, D] over the free dim."""
    sub = 256  # gcd(512, 768)
    nsub = D // sub
    stats = stats_pool.tile([P, nsub, 6], F32, tag="ln_stats")
    tv = t.rearrange("p (n s) -> p n s", s=sub)
    for i in range(nsub):
        nc.vector.bn_stats(out=stats[:, i, :], in_=tv[:, i, :])
    mv = stats_pool.tile([P, 2], F32, tag="ln_mv")
    nc.vector.bn_aggr(out=mv, in_=stats)
    # mv[:,1] = 1/sqrt(var + eps)
    nc.scalar.activation(
        out=mv[:, 1:2], in_=mv[:, 1:2], func=AF.Sqrt, bias=eps_t, scale=1.0
    )
    nc.vector.reciprocal(out=mv[:, 1:2], in_=mv[:, 1:2])
    nc.vector.tensor_scalar(
        out=t,
        in0=t,
        scalar1=mv[:, 0:1],
        scalar2=mv[:, 1:2],
        op0=mybir.AluOpType.subtract,
        op1=mybir.AluOpType.mult,
    )
    nc.vector.tensor_mul(out=t, in0=t, in1=gb)
    nc.vector.tensor_add(out=t, in0=t, in1=bb)


def _emit(nc, tc, io):
    xb, xq, xr = io["xb"], io["xq"], io["xr"]
    ident_d, ones_d, vones_d = io["ident_d"], io["ones_d"], io["vones_d"]
    wq, wk, wv, wo, w1, w2 = (io[k] for k in ("wq", "wk", "wv", "wo", "w1", "w2"))
    bq, bk, b1, b2 = io["bq"], io["bk"], io["b1"], io["b2"]
    g1, bl1, g2, bl2 = io["g1"], io["bl1"], io["g2"], io["bl2"]
    kbias, tril0, tril1, out = io["kbias"], io["tril0"], io["tril1"], io["out"]

    from contextlib import ExitStack

    es = ExitStack()
    const = es.enter_context(tc.tile_pool(name="const", bufs=1))
    ident = const.tile([P, P], F32R)
    nc.sync.dma_start(out=ident, in_=ident_d[:, :])
    eps_t = const.tile([P, 1], F32)
    nc.vector.memset(eps_t, 1e-5)
    ones_t = const.tile([P, DK], F32R)
    nc.sync.dma_start(out=ones_t, in_=_bcast_row(nc, ones_d[:], DK))
    vones_t = const.tile([P, H], F32R)
    nc.sync.dma_start(out=vones_t, in_=_bcast_row(nc, vones_d[:], H))
    bq_t = const.tile([P, KB], F32)
    nc.gpsimd.dma_start(out=bq_t, in_=bq[:].rearrange("(k p) -> p k", p=P))
    bk_t = const.tile([P, KB], F32)
    nc.gpsimd.dma_start(out=bk_t, in_=bk[:].rearrange("(k p) -> p k", p=P))
    b1_t = const.tile([P, MB], F32)
    nc.gpsimd.dma_start(out=b1_t, in_=b1[:].rearrange("(m p) -> p m", p=P))
    g1_t = const.tile([P, D], F32)
    nc.sync.dma_start(out=g1_t, in_=_bcast_row(nc, g1[:], D))
    bl1_t = const.tile([P, D], F32)
    nc.sync.dma_start(out=bl1_t, in_=_bcast_row(nc, bl1[:], D))
    xr_t = const.tile([P, QR // P, D], F32)
    nc.sync.dma_start(out=xr_t, in_=xr[:, :].rearrange("(qb p) d -> p qb d", p=P))
    x1_t = const.tile([P, QR // P, D], F32R)  # post-LN1 activations
    kbias_t = const.tile([P, 2, S // P], F32)
    nc.gpsimd.dma_start(
        out=kbias_t, in_=kbias[:, :].rearrange("c (n p) -> p c n", p=P)
    )
    tril_t = const.tile([P, 2, CH], F32)
    nc.sync.dma_start(out=tril_t[:, 0, :], in_=tril0[:, :])
    nc.sync.dma_start(out=tril_t[:, 1, :], in_=tril1[:, :])

    with (
        tc.tile_pool(name="persist", bufs=1) as persist,
    ):
        kT = persist.tile([P, KB, S], F32R)  # K^T: [d within pb, pb, key]
        qT = persist.tile([P, KB, QR], F32R)  # Q^T
        # V with a ones column per head: [k within block, key block, h*65+e]
        vA = persist.tile([P, S // P, H * (DK + 1)], F32R)
        vA4 = vA.rearrange("p s (h e) -> p s h e", e=DK + 1)
        for s in range(S // P):
            nc.vector.tensor_copy(out=vA4[:, s, :, DK], in_=vones_t)

        # ---------------- Phase 1: QKV projections ----------------
        with (
            tc.tile_pool(name="qkv_x", bufs=3) as xpool,
            tc.tile_pool(name="qkv_ps", bufs=2, space="PSUM") as pps,
            tc.tile_pool(name="qkv_tps", bufs=2, space="PSUM") as tps,
        ):

            def transpose_chunk(src_dram, row0, xt_tile, col0, ncols):
                for sb in range(ncols // P):
                    xtile = xpool.tile([P, D], F32R, tag="xload", name="xload")
                    nc.sync.dma_start(
                        out=xtile,
                        in_=src_dram[row0 + sb * P : row0 + (sb + 1) * P, :],
                    )
                    for k in range(KB):
                        tp = tps.tile([P, P], F32R, tag="tps", name="tps")
                        nc.tensor.transpose(tp, xtile[:, k * P : (k + 1) * P], ident)
                        nc.vector.tensor_copy(
                            out=xt_tile[:, k, col0 + sb * P : col0 + (sb + 1) * P],
                            in_=tp,
                        )

            # Q^T from xq
            with (
                tc.tile_pool(name="q_w", bufs=1) as wqpool,
                tc.tile_pool(name="q_xt", bufs=1) as xqtpool,
            ):
                wq_t = [
                    wqpool.tile([P, D], F32R, tag=f"wq{k}", name=f"wq{k}")
                    for k in range(KB)
                ]
                for k in range(KB):
                    nc.sync.dma_start(out=wq_t[k], in_=wq[k * P : (k + 1) * P, :])
                xqT = xqtpool.tile([P, KB, QR], F32R, tag="xqT", name="xqT")
                transpose_chunk(xq, 0, xqT, 0, QR)
                for pb in range(KB):
                    for n0, nn in _nsplit(QR):
                        ps = pps.tile([P, 512], F32, tag="proj", name="proj")
                        for k in range(KB):
                            nc.tensor.matmul(
                                ps[:, :nn],
                                lhsT=wq_t[k][:, pb * P : (pb + 1) * P],
                                rhs=xqT[:, k, n0 : n0 + nn],
                                start=(k == 0),
                                stop=(k == KB - 1),
                            )
                        nc.vector.tensor_scalar_add(
                            out=qT[:, pb, n0 : n0 + nn],
                            in0=ps[:, :nn],
                            scalar1=bq_t[:, pb : pb + 1],
                        )

            # K^T and V per 512-row key chunk
            with (
                tc.tile_pool(name="kv_w", bufs=1) as wkvpool,
                tc.tile_pool(name="kv_xt", bufs=1) as xbtpool,
            ):
                wk_t = [
                    wkvpool.tile([P, D], F32R, tag=f"wk{k}", name=f"wk{k}")
                    for k in range(KB)
                ]
                wv_t = [
                    wkvpool.tile([P, D], F32R, tag=f"wv{k}", name=f"wv{k}")
                    for k in range(KB)
                ]
                for k in range(KB):
                    nc.sync.dma_start(out=wk_t[k], in_=wk[k * P : (k + 1) * P, :])
                    nc.sync.dma_start(out=wv_t[k], in_=wv[k * P : (k + 1) * P, :])
                for sc in range(S // 512):
                    xbT = xbtpool.tile([P, KB, 512], F32R, tag="xbT", name="xbT")
                    transpose_chunk(xb, sc * 512, xbT, 0, 512)
                    for pb in range(KB):
                        ps = pps.tile([P, 512], F32, tag="proj", name="proj")
                        for k in range(KB):
                            nc.tensor.matmul(
                                ps,
                                lhsT=wk_t[k][:, pb * P : (pb + 1) * P],
                                rhs=xbT[:, k, :],
                                start=(k == 0),
                                stop=(k == KB - 1),
                            )
                        nc.vector.tensor_scalar_add(
                            out=kT[:, pb, sc * 512 : (sc + 1) * 512],
                            in0=ps,
                            scalar1=bk_t[:, pb : pb + 1],
                        )
                    for sb in range(4):
                        ps = pps.tile([P, D], F32, tag="vproj", name="vproj")
                        for k in range(KB):
                            for n0, nn in _nsplit(D):
                                nc.tensor.matmul(
                                    ps[:, n0 : n0 + nn],
                                    lhsT=xbT[:, k, sb * P : (sb + 1) * P],
                                    rhs=wv_t[k][:, n0 : n0 + nn],
                                    start=(k == 0),
                                    stop=(k == KB - 1),
                                )
                        nc.vector.tensor_copy(
                            out=vA4[:, sc * 4 + sb, :, 0:DK],
                            in_=ps.rearrange("p (h d) -> p h d", d=DK),
                        )

        # ---------------- Phase 2: attention ----------------
        with (
            tc.tile_pool(name="att_wo", bufs=1) as wopool,
            tc.tile_pool(name="att_p", bufs=4) as ppool,
            tc.tile_pool(name="att_ctx", bufs=2) as cpool,
            tc.tile_pool(name="att_dn", bufs=2) as dpool,
            tc.tile_pool(name="att_st", bufs=3, space="PSUM") as stps,
            tc.tile_pool(name="att_cx", bufs=1, space="PSUM") as cxps,
            tc.tile_pool(name="att_bc", bufs=1, space="PSUM") as bcps,
            tc.tile_pool(name="att_o", bufs=1, space="PSUM") as ops,
            tc.tile_pool(name="ln_stats", bufs=3) as spool,
        ):
            wo_t = [
                wopool.tile([P, D], F32R, tag=f"wo{k}", name=f"wo{k}")
                for k in range(KB)
            ]
            for k in range(KB):
                nc.sync.dma_start(out=wo_t[k], in_=wo[k * P : (k + 1) * P, :])

            for ch, ext in enumerate(EXT):
                nkb = ext // P
                diag = (0, 1) if ch == 0 else (2, 3)
                ctx = cpool.tile([P, KB, CH], F32R, tag="ctx", name="ctx")
                for h in range(H):
                    pb, base = h // 2, DK * (h % 2)
                    cx = cxps.tile([DK + 1, CH], F32, tag="cx", name="cx")
                    for kb in range(nkb):
                        st = stps.tile([P, CH], F32, tag="st", name="st")
                        nc.tensor.matmul(
                            st,
                            lhsT=kT[base : base + DK, pb, kb * P : (kb + 1) * P],
                            rhs=qT[base : base + DK, pb, ch * CH : (ch + 1) * CH],
                            start=True,
                            stop=True,
                        )
                        if kb in diag:
                            nc.vector.tensor_add(
                                out=st, in0=st, in1=tril_t[:, kb - diag[0], :]
                            )
                        pt = ppool.tile([P, CH], F32R, tag="pt", name="pt")
                        nc.scalar.activation(
                            out=pt,
                            in_=st,
                            func=AF.Exp,
                            scale=0.125,
                            bias=kbias_t[:, ch, kb : kb + 1],
                        )
                        nc.tensor.matmul(
                            cx,
                            lhsT=vA[:, kb, h * (DK + 1) : (h + 1) * (DK + 1)],
                            rhs=pt,
                            start=(kb == 0),
                            stop=(kb == nkb - 1),
                        )
                    dn = dpool.tile([P, CH], F32R, tag="dn", name="dn")
                    nc.vector.tensor_copy(
                        out=dn[DK : DK + 1, :], in_=cx[DK : DK + 1, :]
                    )
                    bc = bcps.tile([DK, CH], F32, tag="bc", name="bc")
                    nc.tensor.matmul(
                        bc,
                        lhsT=ones_t[DK : DK + 1, :],
                        rhs=dn[DK : DK + 1, :],
                        start=True,
                        stop=True,
                    )
                    rc = dpool.tile([DK, CH], F32, tag="rc", name="rc")
                    nc.vector.reciprocal(out=rc, in_=bc)
                    nc.vector.tensor_mul(
                        out=ctx[base : base + DK, pb, :], in0=cx[0:DK, :], in1=rc
                    )
                # O-projection + residual + LN1 per 128-row block
                for qb in range(CH // P):
                    blk = ch * (CH // P) + qb
                    po = ops.tile([P, D], F32, tag="po", name="po")
                    for pb in range(KB):
                        for n0, nn in _nsplit(D):
                            nc.tensor.matmul(
                                po[:, n0 : n0 + nn],
                                lhsT=ctx[:, pb, qb * P : (qb + 1) * P],
                                rhs=wo_t[pb][:, n0 : n0 + nn],
                                start=(pb == 0),
                                stop=(pb == KB - 1),
                            )
                    t = x1_t[:, blk, :]
                    nc.vector.tensor_add(out=t, in0=po, in1=xr_t[:, blk, :])
                    _layernorm(nc, t, g1_t, bl1_t, eps_t, spool)

    # ---------------- Phase 3: FFN + LN2 ----------------
    with (
        tc.tile_pool(name="ffn_xt", bufs=1) as xtpool2,
        tc.tile_pool(name="ffn_h", bufs=1) as hpool,
        tc.tile_pool(name="ffn_w", bufs=1) as wfpool,
        tc.tile_pool(name="ffn_y", bufs=1) as ypool,
        tc.tile_pool(name="ffn_o", bufs=3) as opool,
        tc.tile_pool(name="ffn_tps", bufs=2, space="PSUM") as tps2,
        tc.tile_pool(name="ffn_h_ps", bufs=2, space="PSUM") as hps,
        tc.tile_pool(name="ffn_y_ps", bufs=2, space="PSUM") as yps,
        tc.tile_pool(name="ln_stats2", bufs=3) as spool2,
        tc.tile_pool(name="ffn_const", bufs=1) as fconst,
    ):
        g2_t = fconst.tile([P, D], F32)
        nc.sync.dma_start(out=g2_t, in_=_bcast_row(nc, g2[:], D))
        bl2_t = fconst.tile([P, D], F32)
        nc.sync.dma_start(out=bl2_t, in_=_bcast_row(nc, bl2[:], D))
        b2_t = fconst.tile([P, D], F32)
        nc.sync.dma_start(out=b2_t, in_=_bcast_row(nc, b2[:], D))
        x1T = xtpool2.tile([P, KB, QR], F32R)
        for blk in range(QR // P):
            for k in range(KB):
                tp = tps2.tile([P, P], F32R, tag="tps2", name="tps2")
                nc.tensor.transpose(tp, x1_t[:, blk, k * P : (k + 1) * P], ident)
                nc.vector.tensor_copy(out=x1T[:, k, blk * P : (blk + 1) * P], in_=tp)
        y_acc = ypool.tile([P, QR // P, D], F32, tag="y_acc")
        NH = 2  # ff halves
        FH = DFF // NH
        for half in range(NH):
            w1_t = [
                wfpool.tile([P, FH], F32R, tag=f"w1_{k}", name=f"w1_{k}")
                for k in range(KB)
            ]
            for k in range(KB):
                nc.sync.dma_start(
                    out=w1_t[k],
                    in_=w1[k * P : (k + 1) * P, half * FH : (half + 1) * FH],
                )
            h_t = hpool.tile([P, FH // P, QR], F32R, tag="h", name="h")
            for m in range(FH // P):
                mg = half * (FH // P) + m
                for n0, nn in _nsplit(QR):
                    ph = hps.tile([P, 512], F32, tag="ph", name="ph")
                    for k in range(KB):
                        nc.tensor.matmul(
                            ph[:, :nn],
                            lhsT=w1_t[k][:, m * P : (m + 1) * P],
                            rhs=x1T[:, k, n0 : n0 + nn],
                            start=(k == 0),
                            stop=(k == KB - 1),
                        )
                    nc.scalar.activation(
                        out=h_t[:, m, n0 : n0 + nn],
                        in_=ph[:, :nn],
                        func=AF.Relu,
                        bias=b1_t[:, mg : mg + 1],
                        scale=1.0,
                    )
            w2_t = [
                wfpool.tile([P, D], F32R, tag=f"w2_{k}", name=f"w2_{k}")
                for k in range(FH // P)
            ]
            for k in range(FH // P):
                nc.sync.dma_start(
                    out=w2_t[k],
                    in_=w2[half * FH + k * P : half * FH + (k + 1) * P, :],
                )
            for blk in range(QR // P):
                py = yps.tile([P, D], F32, tag="py", name="py")
                for k in range(FH // P):
                    for n0, nn in _nsplit(D):
                        nc.tensor.matmul(
                            py[:, n0 : n0 + nn],
                            lhsT=h_t[:, k, blk * P : (blk + 1) * P],
                            rhs=w2_t[k][:, n0 : n0 + nn],
                            start=(k == 0),
                            stop=(k == FH // P - 1),
                        )
                if half == 0:
                    nc.vector.tensor_copy(out=y_acc[:, blk, :], in_=py)
                else:
                    t = opool.tile([P, D], F32, tag="obuf", name="obuf")
                    nc.vector.tensor_add(out=t, in0=py, in1=y_acc[:, blk, :])
                    nc.vector.tensor_add(out=t, in0=t, in1=b2_t)
                    nc.vector.tensor_add(out=t, in0=t, in1=x1_t[:, blk, :])
                    _layernorm(nc, t, g2_t, bl2_t, eps_t, spool2)
                    nc.sync.dma_start(out=out[blk * P : (blk + 1) * P, :], in_=t)
    es.close()


# ---------------- host side ----------------


def _numpy_reference(x, mask, Wq, bq, Wk, bk, Wv, bv, Wo, bo, W1, b1, W2, b2,
                     ln1_g, ln1_b, ln2_g, ln2_b):
    def ln(t, g, b, eps=1e-5):
        mu = t.mean(-1, keepdims=True)
        var = t.var(-1, keepdims=True)
        return (t - mu) / np.sqrt(var + eps) * g + b

    b_, s_, d_ = x.shape
    dk = d_ // H

    def split(h):
        return h.reshape(b_, s_, H, dk).transpose(0, 2, 1, 3)

    Q = split(x @ Wq + bq)
    K = split(x @ Wk + bk)
    V = split(x @ Wv + bv)
    sc = np.einsum("bhqd,bhkd->bhqk", Q, K) / np.sqrt(dk)
    sc = np.where(mask == 0, np.float32(-1e9), sc)
    sc = sc - sc.max(-1, keepdims=True)
    p = np.exp(sc)
    p = p / p.sum(-1, keepdims=True)
    ctx = np.einsum("bhqk,bhkd->bhqd", p, V)
    ctx = ctx.transpose(0, 2, 1, 3).reshape(b_, s_, d_)
    x1 = ln(x + ctx @ Wo + bo, ln1_g, ln1_b)
    y = np.maximum(x1 @ W1 + b1, 0.0) @ W2 + b2
    return ln(x1 + y, ln2_g, ln2_b).astype(np.float32)


def _get_program():
    global _PROGRAM
    if _PROGRAM is None:
        _PROGRAM = build_program()
    return _PROGRAM


def _core_rows(c):
    j = c % 4
    return c // 4, np.r_[j * CH : (j + 1) * CH, (7 - j) * CH : (8 - j) * CH]


def _make_in_maps(inputs):
    x = np.asarray(inputs["x"], dtype=np.float32)
    m2 = np.asarray(inputs["mask"]).reshape(S, S)
    maskf = (m2 != 0).astype(np.float32)
    f32 = lambda k: np.ascontiguousarray(np.asarray(inputs[k], dtype=np.float32))
    xr_const = (f32("bv") @ f32("Wo") + f32("bo")).astype(np.float32)

    common = {
        "ident": np.eye(P, dtype=np.float32),
        "ones64": np.ones(DK, dtype=np.float32),
        "vones": np.ones(H, dtype=np.float32),
        "wq": f32("Wq"), "wk": f32("Wk"), "wv": f32("Wv"), "wo": f32("Wo"),
        "w1": f32("W1"), "w2": f32("W2"),
        "bq": f32("bq"), "bk": f32("bk"), "b1": f32("b1"), "b2": f32("b2"),
        "g1": f32("ln1_g"), "bl1": f32("ln1_b"),
        "g2": f32("ln2_g"), "bl2": f32("ln2_b"),
    }
    # constant in-block tril bias tiles (key-relative rows 0-127 / 128-255)
    qi = np.arange(CH)[None, :]
    ki = np.arange(P)[:, None]
    tril0 = np.where(ki <= qi, 0.0, NEG).astype(np.float32)
    tril1 = np.where(ki + P <= qi, 0.0, NEG).astype(np.float32)

    in_maps = []
    for c in range(NCORES):
        b, rows = _core_rows(c)
        j = c % 4
        qa, qb = j * CH, (7 - j) * CH
        rest = np.setdiff1d(np.arange(S), rows)
        perm = np.r_[rows, rest]  # key order: rowsA, rowsB, rest
        orig = perm  # slot -> original key index
        kb_bias = np.full((2, S), NEG, dtype=np.float32)
        # chunk A: active slots = diag rowsA (tril handles) + orig key < qa
        kb_bias[0, 0:CH] = 0.0
        kb_bias[0, 2 * CH :] = np.where(orig[2 * CH :] < qa, 0.0, NEG)
        # chunk B: diag rowsB + orig key < qb
        kb_bias[1, CH : 2 * CH] = 0.0
        kb_bias[1, 0:CH] = 0.0  # rowsA keys are always < qb
        kb_bias[1, 2 * CH :] = np.where(orig[2 * CH :] < qb, 0.0, NEG)
        xq_c = np.ascontiguousarray(x[b][rows])
        in_maps.append(
            {
                "xb": np.ascontiguousarray(x[b][perm]),
                "xq": xq_c,
                "xr": (xq_c + xr_const).astype(np.float32),
                "kbias": kb_bias,
                "tril0": tril0,
                "tril1": tril1,
                **common,
            }
        )
    return in_maps


def kernel(x, mask, Wq, bq, Wk, bk, Wv, bv, Wo, bo, W1, b1, W2, b2,
           ln1_g, ln1_b, ln2_g, ln2_b):
    inputs = dict(x=x, mask=mask, Wq=Wq, bq=bq, Wk=Wk, bk=bk, Wv=Wv, bv=bv,
                  Wo=Wo, bo=bo, W1=W1, b1=b1, W2=W2, b2=b2, ln1_g=ln1_g,
                  ln1_b=ln1_b, ln2_g=ln2_g, ln2_b=ln2_b)
    x = np.asarray(x, dtype=np.float32)
    m2 = np.asarray(mask).reshape(-1)
    ok = x.shape == (B, S, D) and np.asarray(mask).size == S * S and np.array_equal(
        np.asarray(mask).reshape(S, S) != 0,
        np.tril(np.ones((S, S), dtype=bool)),
    )
    if not ok:
        args = [np.asarray(a, dtype=np.float32) for a in
                (Wq, bq, Wk, bk, Wv, bv, Wo, bo, W1, b1, W2, b2,
                 ln1_g, ln1_b, ln2_g, ln2_b)]
        return _numpy_reference(x, np.asarray(mask), *args)

    nc = _get_program()
    in_maps = _make_in_maps(inputs)
    res = run_bass_kernel_spmd(nc, in_maps, list(range(NCORES)))
    outp = np.empty((B, S, D), dtype=np.float32)
    for c in range(NCORES):
        b, rows = _core_rows(c)
        outp[b][rows] = res.results[c]["out"]
    return outp
